# revision 46
# baseline (speedup 1.0000x reference)
"""Trainium2 raw-Bass kernel for nn_InteractionPruningLayer (sparse_attention).

Math (B=1024, F=256, D=64):
    qkv   = einsum('fd,nde->nfe', indicator, W_qkv)            # [3,F,D]
    gate  = (trans[0] @ trans[1].T > 0);  G = (qkv1 @ qkv0.T) * gate
    s[n,b,f] = feature[b,f,:] . qkv[n,f,:];  t = s0*s2;  u = s1
    out[b,i,:] = t[b,i] * sum_j u[b,j] * G[i,j] * qkv2[j,:]

Split of work:
    host   — weight prep (K2[j,i,d] = G[i,j]*qkv2[j,d]), the per-(b,f)
             projections t/u, and the t-scale for the 16 features/core the
             scalar engine drains (~2% of FLOPs total)
    device — 8 cores sharded along the FEATURE dim i (32 features/core):
             each core contracts the full batch against its own 1MB slice
             of K2 (536M MACs/core, 98% of FLOPs):
                 psum[b,(i,d)] = sum_j uT[j,b] * w[j,(i,d)]    (PE, bf16)
                 out[b,(i,d)]  = t[b,i] * psum[b,(i,d)]        (drain)
             i-sharding means K2 is *sharded*, not replicated: no on-chip
             G-broadcast build and tiny input DMAs.

Engine orchestration:
  - K2 is never shipped: a 48KB pack (GT-shard + qkv2) is DMA'd and the
    vector engine expands K2[j,(i,d)] = GT[j,i]*qkv2[j,d] on-chip with
    four double-broadcast tensor_muls, in matmul consumption order
  - input DMAs spread over the three DMA-capable rings (sync: gq + u1,
    scalar: u0, gpsimd: t + u2 + u3), u packed batch-chunk-major and
    streamed in four 256KB chunks with one semaphore each (DMA
    completions arrive as 16 per-stripe +1 increments, so a shared
    semaphore across chunks would gate unsoundly)
  - 4 PSUM slots [128,1024]; each batch-chunk is two jc-accumulated
    halves, so slot-reuse distance (~3.5us of PE work) absorbs the
    ~0.5-1.3us semaphore hop latency and the PE never waits on drains
  - PSUM drain split: vector does cols 0:1024 with the fused t-multiply
    (tensor_tensor); scalar does cols 1024:2048 as a plain copy-cast
    (ACT cannot tensor-multiply; host applies t for those 16 features)
  - output leaves as 16 quarter-row DMAs balanced 6/5/5 over the
    gpsimd/scalar/sync rings so the tail only waits on one 256KB piece

Cross-execution semaphore safety: sem state survives across NEFF
executions on these long-lived axon terminals. The kernel postamble
clears the whole sem range, so only the FIRST execution after a foreign
NEFF sees garbage. Defenses: (1) gpsimd dma_reset + sem_clear over the
whole kernel range first, (2) every engine re-clears each semaphore it
waits on before its first wait (clear->wait on the same engine is
race-free), and the framework's init all-engine barrier plus >=4us DMA
latency guarantees no completion increment can precede those clears.
"""

import numpy as np
import ml_dtypes

B, F, D = 1024, 256, 64
NCORES = 8
ISH = F // NCORES          # 32 features per core
CW = ISH * D               # 2048 psum/output columns per core
NBC = B // 128             # 8 batch chunks of 128 rows
VC = 1024                  # vector-drain cols (PSUM-bank aligned: 2x512)
HI = VC // D               # first host-side feature index of the raw slice
_compiled = None


def _setup_jax_cache():
    import jax
    try:
        if jax.config.jax_compilation_cache_dir is None:
            jax.config.update("jax_compilation_cache_dir",
                              "/tmp/bass_jax_cache")
            jax.config.update("jax_persistent_cache_min_entry_size_bytes", -1)
            jax.config.update("jax_persistent_cache_min_compile_time_secs", 0)
    except Exception:
        pass


def _host_precompute(indicator, W_qk, W_qkv):
    """Returns gq [128,2,F/4+D] f32 packed (jl, jc, i|d) and qkv [3,F,D].

    gq[:, jc, 0:F] is GT (gate-masked cross weights); gq[:, jc, F:F+D] is
    qkv2. The device expands their outer product K2[j,(i,d)] on-chip."""
    ind = np.asarray(indicator, dtype=np.float32)
    qkv = np.einsum('fd,nde->nfe', ind, np.asarray(W_qkv, dtype=np.float32))
    trans = np.einsum('fd,nde->nfe', ind, np.asarray(W_qk, dtype=np.float32))
    gate = (trans[0] @ trans[1].T) > 0
    G = np.where(gate, qkv[1] @ qkv[0].T, np.float32(0.0)).astype(np.float32)
    gts = G.T.reshape(2, 128, F).transpose(1, 0, 2)        # [jl, jc, i]
    qv = qkv[2].reshape(2, 128, D).transpose(1, 0, 2)      # [jl, jc, d]
    gq = np.concatenate([gts, np.broadcast_to(qv, qv.shape)], axis=2)
    return np.ascontiguousarray(gq), qkv


def _host_tu(feature, qkv):
    """t = s0*s2 (f32 [B,F]), uT packed [jl, bc, jc, bl] bf16 [128,8,2,128]."""
    f = np.asarray(feature, dtype=np.float32)
    s = np.einsum('bfd,nfd->nbf', f, qkv, optimize=True)
    t = (s[0] * s[2]).astype(np.float32)                   # [B, F]
    u = s[1].astype(ml_dtypes.bfloat16)                    # [B, F]
    uT = np.ascontiguousarray(                             # [jl, bc, jc, bl]
        u.T.reshape(2, 128, NBC, 128).transpose(1, 2, 0, 3))
    return t, uT


def _build_bass():
    import concourse.bass as bass
    from concourse import mybir

    nc = bass.Bass()
    f32, bf16 = mybir.dt.float32, mybir.dt.bfloat16

    u_d = nc.declare_dram_parameter("uT", [128, 2 * B], bf16, isOutput=False)
    gq_d = nc.declare_dram_parameter("gq", [128, 2 * (ISH + D)], bf16,
                                     isOutput=False)
    t_d = nc.declare_dram_parameter("tvec", [128, NBC * ISH], f32, isOutput=False)
    out_d = nc.declare_dram_parameter("out", [B, CW], bf16, isOutput=True)

    u_sb = nc.alloc_sbuf_tensor("u_sb", [128, 2 * B], bf16).ap()
    gq_sb = nc.alloc_sbuf_tensor("gq_sb", [128, 2 * (ISH + D)], bf16).ap()
    w_sb = nc.alloc_sbuf_tensor("w_sb", [128, 2 * CW], bf16).ap()
    t_sb = nc.alloc_sbuf_tensor("t_sb", [128, NBC * ISH], f32).ap()
    osb = nc.alloc_sbuf_tensor("osb", [128, NBC * CW], bf16).ap()
    grep = nc.alloc_sbuf_tensor("grep", [128, 2 * VC], bf16).ap()
    mp = [nc.alloc_psum_tensor(f"mp{i}", [128, VC], f32).ap() for i in range(4)]

    u2 = u_sb.rearrange("k (c j b) -> k c j b", c=NBC, j=2)  # [jl, bc, jc, bl]
    w2 = w_sb.rearrange("k (c x) -> k c x", c=2)           # [jl, jc, (i d)]
    gq2 = gq_sb.rearrange("k (c x) -> k c x", c=2)         # [jl, jc, (i|d)]
    t3 = t_sb.rearrange("b (c i) -> b c i", c=NBC)         # [bl, bc, il]

    NUC = 4                    # u DMA chunks (2 batch-chunks each)
    UCW = 2 * B // NUC         # 512 u columns per chunk

    with (
        nc.Block(no_gpsimd_drain=True) as block,
        nc.semaphore("sU0") as sU0,
        nc.semaphore("sU1") as sU1,
        nc.semaphore("sU2") as sU2,
        nc.semaphore("sU3") as sU3,
        nc.semaphore("sGQ") as sGQ,
        nc.semaphore("sG2") as sG2,
        nc.semaphore("sK") as sK,
        nc.semaphore("sT") as sT,
        nc.semaphore("sMv") as sMv,
        nc.semaphore("sMc") as sMc,
        nc.semaphore("sE") as sE,
        nc.semaphore("sC") as sC,
        nc.semaphore("sO") as sO,
    ):
        sU = [sU0, sU1, sU2, sU3]

        @block.gpsimd
        def _(g):
            g.sem_clear(sE)
            g.sem_clear(sO)
            g.dma_start(out=t_sb[:], in_=t_d[:]).then_inc(sT, 16)
            for k in range(2, NUC):
                g.dma_start(out=u_sb[:, UCW * k:UCW * (k + 1)],
                            in_=u_d[:, UCW * k:UCW * (k + 1)]
                            ).then_inc(sU[k], 16)
            for bc in range(NBC - 1):
                g.wait_ge(sE, bc + 1)
                g.dma_start(out=out_d[128 * bc:128 * (bc + 1), 0:VC],
                            in_=osb[:, CW * bc:CW * bc + VC]).then_inc(sO, 16)
            g.wait_ge(sO, 32 * NBC)

        @block.sync
        def _(sp):
            sp.sem_clear(sC)
            sp.sem_clear(sE)
            sp.dma_start(out=gq_sb[:], in_=gq_d[:]).then_inc(sGQ, 16)
            sp.dma_start(out=u_sb[:, UCW:2 * UCW],
                         in_=u_d[:, UCW:2 * UCW]).then_inc(sU1, 16)
            for bc in range(0, NBC, 2):
                sp.wait_ge(sC, bc + 1)
                sp.dma_start(out=out_d[128 * bc:128 * (bc + 1), VC:CW],
                             in_=osb[:, CW * bc + VC:CW * (bc + 1)]
                             ).then_inc(sO, 16)
            sp.wait_ge(sE, NBC)
            sp.dma_start(out=out_d[128 * (NBC - 1):128 * NBC, 0:VC],
                         in_=osb[:, CW * (NBC - 1):CW * (NBC - 1) + VC]
                         ).then_inc(sO, 16)

        @block.scalar
        def _(a):
            a.sem_clear(sMc)
            a.dma_start(out=u_sb[:, 0:UCW],
                        in_=u_d[:, 0:UCW]).then_inc(sU0, 16)
            # warm the ACT pipeline (absorbs the ~1-2us first-op latency)
            a.copy(out=osb[0:1, 0:1], in_=osb[0:1, 0:1])
            a.sem_clear(sGQ)
            a.wait_ge(sGQ, 16)
            HV2 = VC // D
            for gg in range(2):
                # jc1 GT-broadcast materialize: vector multiplies these as
                # a dense operand (2x DVE rate vs double-broadcast)
                a.copy(out=grep[:, VC * gg:VC * (gg + 1)].rearrange(
                           "k (i d) -> k i d", d=D),
                       in_=gq2[:, 1, HV2 * gg:HV2 * (gg + 1)].unsqueeze(2)
                       .broadcast_to([128, HV2, D])).then_inc(sG2, 1)
            for bc in range(NBC):
                a.wait_ge(sMc, bc + 1)
                a.copy(out=osb[:, CW * bc + VC:CW * (bc + 1)],
                       in_=mp[(2 * bc + 1) % 4]).then_inc(sC, 1)
                if bc % 2 == 1:
                    # self-wait: the copy's writes must commit before the
                    # DMA engine reads osb (then_inc fires on completion)
                    a.wait_ge(sC, bc + 1)
                    a.dma_start(out=out_d[128 * bc:128 * (bc + 1), VC:CW],
                                in_=osb[:, CW * bc + VC:CW * (bc + 1)]
                                ).then_inc(sO, 16)

        @block.vector
        def _(v):
            v.sem_clear(sGQ)
            v.sem_clear(sT)
            v.sem_clear(sMv)
            v.sem_clear(sG2)
            v.wait_ge(sGQ, 16)
            HV = VC // D
            for g in range(2):
                # K2 build jc0: w[jl,(i,d)] = GT[jl,i] * qkv2[jl,d]
                v.tensor_mul(
                    w2[:, 0, VC * g:VC * (g + 1)].rearrange(
                        "k (i d) -> k i d", d=D),
                    gq2[:, 0, ISH:ISH + D].unsqueeze(1)
                    .broadcast_to([128, HV, D]),
                    gq2[:, 0, HV * g:HV * (g + 1)].unsqueeze(2)
                    .broadcast_to([128, HV, D]),
                ).then_inc(sK, 1)
            for g in range(2):
                # K2 build jc1 from the scalar-materialized dense GT
                v.wait_ge(sG2, g + 1)
                v.tensor_mul(
                    w2[:, 1, VC * g:VC * (g + 1)].rearrange(
                        "k (i d) -> k i d", d=D),
                    grep[:, VC * g:VC * (g + 1)].rearrange(
                        "k (i d) -> k i d", d=D),
                    gq2[:, 1, ISH:ISH + D].unsqueeze(1)
                    .broadcast_to([128, HV, D]),
                ).then_inc(sK, 1)
            v.wait_ge(sT, 16)
            for bc in range(NBC):
                v.wait_ge(sMv, bc + 1)
                v.tensor_mul(
                    osb[:, CW * bc:CW * bc + VC].rearrange(
                        "b (i d) -> b i d", d=D),
                    mp[(2 * bc) % 4].rearrange("b (i d) -> b i d", d=D),
                    t3[:, bc, 0:HI].unsqueeze(2).broadcast_to([128, HI, D]),
                ).then_inc(sE, 1)

        @block.tensor
        def _(t):
            for s in (sU0, sU1, sU2, sU3, sK, sE, sC):
                t.sem_clear(s)
            t.wait_ge(sU0, 16)
            t.wait_ge(sK, 1)
            for bc in range(NBC):
                if bc % 2 == 0 and bc > 0:
                    t.wait_ge(sU[bc // 2], 16)
                for jc in range(2):
                    for g in range(2):
                        # half g covers cols [g*VC, (g+1)*VC) of chunk bc
                        if bc == 0:
                            t.wait_ge(sK, 2 * jc + g + 1)
                        if bc >= 2 and jc == 0:
                            t.wait_ge(sE if g == 0 else sC, bc - 1)
                        for h in range(2):
                            mm = t.matmul(
                                out=mp[(2 * bc + g) % 4][:,
                                                         512 * h:512 * (h + 1)],
                                lhsT=u2[:, bc, jc, :],
                                rhs=w2[:, jc, VC * g + 512 * h:
                                       VC * g + 512 * (h + 1)],
                                start=(jc == 0), stop=(jc == 1))
                        if jc == 1:
                            mm.then_inc(sMv if g == 0 else sMc, 1)

    return nc


def _make_in_maps(feature, gq, qkv):
    t, uT = _host_tu(feature, qkv)
    # t packed per core: t_d[p, bc*ISH + il] = t[bc*128 + p, c*ISH + il]
    tp = t.reshape(NBC, 128, F).transpose(1, 0, 2)         # [p, bc, F]
    u_flat = np.ascontiguousarray(uT.reshape(128, 2 * B))
    in_maps = []
    for c in range(NCORES):
        gqc = np.concatenate(
            [gq[:, :, c * ISH:(c + 1) * ISH], gq[:, :, F:F + D]], axis=2)
        gqc = np.ascontiguousarray(
            gqc.reshape(128, 2 * (ISH + D))).astype(ml_dtypes.bfloat16)
        tc = np.ascontiguousarray(
            tp[:, :, c * ISH:(c + 1) * ISH].reshape(128, NBC * ISH))
        in_maps.append({"uT": u_flat, "gq": gqc, "tvec": tc})
    return in_maps, t


def kernel(feature, indicator, W_qk, W_qkv):
    global _compiled
    _setup_jax_cache()
    from concourse.bass_utils import run_bass_kernel_spmd

    gq, qkv = _host_precompute(indicator, W_qk, W_qkv)
    if _compiled is None:
        _compiled = _build_bass()
    nc = _compiled

    in_maps, t = _make_in_maps(feature, gq, qkv)
    res = run_bass_kernel_spmd(nc, in_maps, list(range(NCORES)))
    out = np.empty((B, F, D), dtype=np.float32)
    for c in range(NCORES):
        oc = res.results[c]["out"].astype(np.float32).reshape(B, ISH, D)
        # cols HI..ISH came from the scalar copy-cast drain: apply t here
        oc[:, HI:, :] *= t[:, c * ISH + HI:(c + 1) * ISH, None]
        out[:, c * ISH:(c + 1) * ISH, :] = oc
    return out
